# revision 1
# baseline (speedup 1.0000x reference)
"""Trainium2 Bass kernel for nn_CrossAttentionBlock_85349590106342.

Shape contract (hardcoded, from the problem spec):
  x: [8, 6144, 256] f32; 4 linears [256,256]; LayerNorm(256); heads=8.
  The 6144 seq dim is 3 concatenated streams of n=2048: origin|emap|lbp.
  out[b] = attn(q=LN(origin)Wq^T, kv=LN(emap))Wo^T + bo
         + attn(q=LN(origin)Wq^T, kv=LN(lbp))Wo^T + bo

Sharding: data-parallel over batch b across the 8 cores (one batch element
per core); all params replicated. No collectives.

Per-core dataflow:
  - LayerNorm row-major (bn_stats/bn_aggr + Sqrt + reciprocal), LN gamma/beta
    and the 1/sqrt(dim) score scale are folded into the weights host-side.
  - xn cast to bf16, DMA-xbar-transposed to feature-major xnT [256, 6144].
  - qT/kT projections (transposed outputs, feature on partitions) and
    row-major v (kv on partitions), all bf16 matmuls w/ bias via K=1 matmul.
  - scores computed transposed S^T[kv, q] so the AV matmul needs no
    transposes: 4 heads packed via PE row-tiling (K=32 strips).
  - exp on ScalarE in [128, 2048] ops spanning 4 PSUM banks (4 heads).
  - AV + softmax denominators in one matmul via ones-augmented [v|1] lhsT
    (M=33), 2 heads col-tiled concurrently, accumulated over kv in PSUM.
  - normalize after AV: reciprocal of sums, broadcast across partitions via
    a tiny K=2 float32r matmul, multiply -> attn_outT bf16.
  - out-projection uses attn_outT directly as lhsT to produce row-major
    out[i, j], accumulating both passes + bias in PSUM.
"""

import numpy as np
import ml_dtypes

import concourse.bass as bass
import concourse.tile as tile
from concourse import bacc, mybir
from concourse.bass_utils import run_bass_kernel_spmd
from contextlib import ExitStack

F32 = mybir.dt.float32
F32R = mybir.dt.float32r
BF16 = mybir.dt.bfloat16
AF = mybir.ActivationFunctionType
ALU = mybir.AluOpType

P = 128          # partitions
DIM = 256        # model dim
NSTREAM = 2048   # tokens per stream
NTOT = 6144      # 3 streams
NROWT = NTOT // P   # 48 row tiles for LN
H = 8            # heads
D = 32           # head dim
QC = 512         # q chunk (columns per scores matmul)
NQC = NSTREAM // QC  # 4
KVC = NSTREAM // P   # 16 kv chunks
LN_EPS = 1e-5

_CACHED_NC = None


def _build_nc():
    nc = bacc.Bacc("TRN2", target_bir_lowering=False, debug=False)

    x_h = nc.dram_tensor("x", [NTOT, DIM], F32, kind="ExternalInput")
    w_h = {}
    b_h = {}
    for nm in ("wq", "wk", "wv", "wo"):
        w_h[nm] = nc.dram_tensor(nm, [DIM, DIM], BF16, kind="ExternalInput")
    for nm in ("bq", "bk", "bv", "bo"):
        b_h[nm] = nc.dram_tensor(nm, [1, DIM], BF16, kind="ExternalInput")
    e64_h = nc.dram_tensor("e64", [2, 64], F32R, kind="ExternalInput")
    out_h = nc.dram_tensor("out", [NSTREAM, DIM], F32, kind="ExternalOutput")

    with tile.TileContext(nc) as tc, ExitStack() as ctx:
        _body(ctx, tc, x_h, w_h, b_h, e64_h, out_h)
    nc.compile()
    return nc


def _body(ctx, tc, x_h, w_h, b_h, e64_h, out_h):
    nc = tc.nc

    consts = ctx.enter_context(tc.tile_pool(name="consts", bufs=1))
    w_sb = {}
    for nm in ("wq", "wk", "wv", "wo"):
        w_sb[nm] = []
        for kc in range(2):
            t = consts.tile([P, DIM], BF16, tag=f"{nm}{kc}", name=f"{nm}{kc}")
            nc.sync.dma_start(out=t[:], in_=w_h[nm].ap()[P * kc : P * (kc + 1), :])
            w_sb[nm].append(t)
    b_sb = {}
    for nm in ("bq", "bk", "bv", "bo"):
        t = consts.tile([1, DIM], BF16, tag=nm, name=nm)
        nc.sync.dma_start(out=t[:], in_=b_h[nm].ap()[:, :])
        b_sb[nm] = t
    e64 = consts.tile([2, 64], F32R, tag="e64")
    nc.sync.dma_start(out=e64[:], in_=e64_h.ap()[:, :])
    ones_row = consts.tile([1, QC], BF16, tag="ones")
    nc.vector.memset(ones_row[:], 1.0)
    eps_t = consts.tile([P, 1], F32, tag="eps")
    nc.vector.memset(eps_t[:], LN_EPS)
    zero_col = consts.tile([1, P], BF16, tag="zeroc")
    nc.vector.memset(zero_col[:], 0.0)
    ones32 = consts.tile([1, 32], BF16, tag="ones32")
    nc.vector.memset(ones32[:], 1.0)

    # persistent feature-major normalized input
    xnt_pool = ctx.enter_context(tc.tile_pool(name="xnt", bufs=1))
    xnT = [xnt_pool.tile([P, NTOT], BF16, tag=f"xnt{kc}", name=f"xnt{kc}") for kc in range(2)]

    # ---------------- Phase 1: LayerNorm + transpose ----------------
    with tc.tile_pool(name="lnx", bufs=4) as lnx, tc.tile_pool(name="lnst", bufs=8) as lst:
        for t in range(NROWT):
            xt = lnx.tile([P, DIM], F32, tag="x")
            nc.sync.dma_start(out=xt[:], in_=x_h.ap()[P * t : P * (t + 1), :])
            st = lst.tile([P, 6], F32, tag="st")
            nc.vector.bn_stats(out=st[:], in_=xt[:])
            mv = lst.tile([P, 2], F32, tag="mv")
            nc.vector.bn_aggr(out=mv[:], in_=st[:])
            sig = lst.tile([P, 1], F32, tag="sig")
            nc.scalar.activation(out=sig[:], in_=mv[:, 1:2], func=AF.Sqrt,
                                 bias=eps_t[:], scale=1.0)
            rs = lst.tile([P, 1], F32, tag="rs")
            nc.vector.reciprocal(out=rs[:], in_=sig[:])
            xn = lnx.tile([P, DIM], BF16, tag="xn")
            nc.vector.tensor_scalar(out=xn[:], in0=xt[:],
                                    scalar1=mv[:, 0:1], scalar2=rs[:],
                                    op0=ALU.subtract, op1=ALU.mult)
            for kc in range(2):
                nc.sync.dma_start_transpose(
                    out=xnT[kc][:, P * t : P * (t + 1)],
                    in_=xn[:, P * kc : P * (kc + 1)],
                )

    # ---------------- Phase 2: projections ----------------
    qkv = ctx.enter_context(tc.tile_pool(name="qkv", bufs=1))
    qT = [qkv.tile([P, NSTREAM], BF16, tag=f"qT{g}", name=f"qT{g}") for g in range(2)]
    kT = {}
    for s in ("e", "l"):
        for g in range(2):
            kT[(s, g)] = qkv.tile([P, NSTREAM], BF16, tag=f"kT{s}{g}", name=f"kT{s}{g}")
    vaug = {}
    for s in ("e", "l"):
        for c in range(KVC):
            vaug[(s, c)] = qkv.tile([P, 33 * H], BF16, tag=f"v{s}{c}", name=f"v{s}{c}")

    sbase = {"q": 0, "e": NSTREAM, "l": 2 * NSTREAM}

    with tc.tile_pool(name="pps", bufs=2, space="PSUM") as pps:
        # qT and kT (transposed projections)
        for (dst_list, wname, bname, stream) in (
            ((qT[0], qT[1]), "wq", "bq", "q"),
            ((kT[("e", 0)], kT[("e", 1)]), "wk", "bk", "e"),
            ((kT[("l", 0)], kT[("l", 1)]), "wk", "bk", "l"),
        ):
            base = sbase[stream]
            for g in range(2):
                for c4 in range(NQC):
                    ps = pps.tile([P, QC], F32, tag="pp")
                    for kc in range(2):
                        nc.tensor.matmul(
                            out=ps[:],
                            lhsT=w_sb[wname][kc][:, P * g : P * (g + 1)],
                            rhs=xnT[kc][:, base + QC * c4 : base + QC * (c4 + 1)],
                            start=(kc == 0), stop=False,
                        )
                    nc.tensor.matmul(
                        out=ps[:],
                        lhsT=b_sb[bname][:, P * g : P * (g + 1)],
                        rhs=ones_row[:],
                        start=False, stop=True,
                    )
                    nc.any.tensor_copy(
                        out=dst_list[g][:, QC * c4 : QC * (c4 + 1)], in_=ps[:]
                    )
        # v (row-major, ones-augmented layout [128, 8*33])
        for s in ("e", "l"):
            base = sbase[s]
            for c in range(KVC):
                ps = pps.tile([P, DIM], F32, tag="pp", name="vps")
                for kc in range(2):
                    nc.tensor.matmul(
                        out=ps[:],
                        lhsT=xnT[kc][:, base + P * c : base + P * (c + 1)],
                        rhs=w_sb["wv"][kc][:],
                        start=(kc == 0), stop=False,
                    )
                nc.tensor.matmul(
                    out=ps[:],
                    lhsT=ones_row[:, 0:P],
                    rhs=b_sb["bv"][:],
                    start=False, stop=True,
                )
                vt = vaug[(s, c)]
                vt_r = vt[:].rearrange("p (h w) -> p h w", w=33)
                nc.vector.memset(vt_r[:, :, 32:33], 1.0)
                nc.any.tensor_copy(
                    out=vt_r[:, :, 0:32],
                    in_=ps[:].rearrange("p (h w) -> p h w", w=32),
                )

    # ---------------- Phase 3: attention + out-proj ----------------
    avps_ctx = tc.tile_pool(name="avps", bufs=1, space="PSUM")
    scps_ctx = tc.tile_pool(name="scps", bufs=1, space="PSUM")
    avps_pool = ctx.enter_context(avps_ctx)
    scps_pool = ctx.enter_context(scps_ctx)
    exp_pool = ctx.enter_context(tc.tile_pool(name="expp", bufs=2))
    rec_pool = ctx.enter_context(tc.tile_pool(name="rec", bufs=2))
    bc_pool = ctx.enter_context(tc.tile_pool(name="bc", bufs=2))
    ao_pool = ctx.enter_context(tc.tile_pool(name="ao", bufs=2))
    ost_pool = ctx.enter_context(tc.tile_pool(name="ost", bufs=3))

    for qc in range(NQC):
        ao = {}
        for s in ("e", "l"):
            avp = avps_pool.tile([P, 4 * QC], F32, tag="av")
            for bank in range(4):
                nc.tensor.matmul(
                    out=avp[:, QC * bank : QC * (bank + 1)],
                    lhsT=zero_col[:],
                    rhs=ones_row[:],
                    start=True, stop=False,
                    skip_group_check=True,
                )
            for c in range(KVC):
                exs = []
                for g in range(2):
                    scp = scps_pool.tile([P, 4 * QC], F32, tag="sc")
                    for hl in range(4):
                        h = 4 * g + hl
                        nc.tensor.matmul(
                            out=scp[:, QC * hl : QC * (hl + 1)],
                            lhsT=kT[(s, g)][D * hl : D * (hl + 1), P * c : P * (c + 1)],
                            rhs=qT[g][D * hl : D * (hl + 1), QC * qc : QC * (qc + 1)],
                            start=True, stop=True,
                            tile_position=(D * hl, 0),
                        )
                    ex = exp_pool.tile([P, 4 * QC], BF16, tag="exp")
                    nc.scalar.activation(out=ex[:], in_=scp[:], func=AF.Exp)
                    exs.append(ex)
                # AV + sums, after both exps are issued: col-tiled pairs
                for g in range(2):
                    for hl in range(4):
                        h = 4 * g + hl
                        pr, par = h // 2, h % 2
                        nc.tensor.matmul(
                            out=avp[64 * par : 64 * par + 33,
                                    QC * pr : QC * (pr + 1)],
                            lhsT=vaug[(s, c)][:, 33 * h : 33 * h + 33],
                            rhs=exs[g][:, QC * hl : QC * (hl + 1)],
                            start=False, stop=False,
                            skip_group_check=True,
                            tile_position=(0, 64 * par),
                        )
            # softmax denominators -> reciprocal (32-aligned partition bases)
            rc0 = rec_pool.tile([1, 4 * QC], BF16, tag="rc0", name="rc0")
            rc1 = rec_pool.tile([1, 4 * QC], BF16, tag="rc1", name="rc1")
            with nc.allow_low_precision(reason="softmax denom recip in bf16"):
                nc.vector.reciprocal(out=rc0[:], in_=avp[32:33, :])
                nc.vector.reciprocal(out=rc1[:], in_=avp[96:97, :])
            rcs = (rc0, rc1)
            # normalize: broadcast recip across partitions, multiply
            for g in range(2):
                aot = ao_pool.tile([P, QC], BF16, tag=f"ao{s}{g}", name=f"ao{s}{g}")
                for pl in range(2):
                    pr = 2 * g + pl
                    bcp = scps_pool.tile([P, QC], F32, tag="sc")
                    for par in range(2):
                        nc.tensor.matmul(
                            out=bcp[32 * par : 32 * (par + 1), :],
                            lhsT=ones32[:],
                            rhs=rcs[par][:, QC * pr : QC * (pr + 1)],
                            start=True, stop=True, skip_group_check=True,
                        )
                    bcs = bc_pool.tile([64, QC], F32, tag="bcs")
                    nc.vector.tensor_copy(out=bcs[:], in_=bcp[0:64, :])
                    for par in range(2):
                        nc.vector.tensor_mul(
                            out=aot[64 * pl + 32 * par : 64 * pl + 32 * par + 32, :],
                            in0=avp[64 * par : 64 * par + 32,
                                    QC * pr : QC * (pr + 1)],
                            in1=bcs[32 * par : 32 * par + 32, :],
                        )
                ao[(s, g)] = aot
        # out-projection for this q chunk (both passes accumulated)
        for isub in range(4):
            op = scps_pool.tile([P, DIM], F32, tag="sc")
            first = True
            for s in ("e", "l"):
                for g in range(2):
                    nc.tensor.matmul(
                        out=op[:],
                        lhsT=ao[(s, g)][:, P * isub : P * (isub + 1)],
                        rhs=w_sb["wo"][g][:],
                        start=first, stop=False,
                    )
                    first = False
            nc.tensor.matmul(
                out=op[:],
                lhsT=ones_row[:, 0:P],
                rhs=b_sb["bo"][:],
                start=False, stop=True,
            )
            ost = ost_pool.tile([P, DIM], F32, tag="ost")
            nc.vector.tensor_copy(out=ost[:], in_=op[:])
            nc.sync.dma_start(
                out=out_h.ap()[QC * qc + P * isub : QC * qc + P * (isub + 1), :],
                in_=ost[:],
            )


def _get_nc():
    global _CACHED_NC
    if _CACHED_NC is None:
        _CACHED_NC = _build_nc()
    return _CACHED_NC


def _prep_host_inputs(x, ln_g, ln_b, Wq, bq, Wk, bk, Wv, bv, Wo, bo):
    """Fold LN affine + score scale into weights; build per-core input maps."""
    bf = ml_dtypes.bfloat16
    g = ln_g.astype(np.float64)
    b = ln_b.astype(np.float64)
    scale = 1.0 / 16.0  # dim ** -0.5

    def eff(W, bias, s):
        W = W.astype(np.float64)
        bias = bias.astype(np.float64)
        Weff = (W * g[None, :]) * s          # q = z @ (W*g).T + (W@b + bias)
        beff = (W @ b + bias) * s
        return Weff.T.astype(bf), beff.reshape(1, DIM).astype(bf)

    wqT, bq_e = eff(Wq, bq, scale)
    wkT, bk_e = eff(Wk, bk, 1.0)
    wvT, bv_e = eff(Wv, bv, 1.0)
    woT = Wo.astype(np.float64).T.astype(bf)
    bo_e = (2.0 * bo.astype(np.float64)).reshape(1, DIM).astype(bf)  # bo added in both passes

    e64 = np.zeros((2, 64), np.float32)
    for r in range(2):
        e64[r, 32 * r : 32 * (r + 1)] = 1.0

    common = {
        "wq": wqT, "wk": wkT, "wv": wvT, "wo": woT,
        "bq": bq_e, "bk": bk_e, "bv": bv_e, "bo": bo_e,
        "e64": e64,
    }
    in_maps = []
    for i in range(x.shape[0]):
        m = dict(common)
        m["x"] = np.ascontiguousarray(x[i].astype(np.float32))
        in_maps.append(m)
    return in_maps


def run(inputs, trace=False, **spmd_kwargs):
    """Run on hardware; returns (out [8,2048,256] f32, BassKernelResults)."""
    x = np.asarray(inputs["x"], np.float32)
    assert x.shape == (8, NTOT, DIM), x.shape
    assert int(inputs["heads"]) == H
    in_maps = _prep_host_inputs(
        x,
        np.asarray(inputs["ln_g"], np.float32), np.asarray(inputs["ln_b"], np.float32),
        np.asarray(inputs["Wq"], np.float32), np.asarray(inputs["bq"], np.float32),
        np.asarray(inputs["Wk"], np.float32), np.asarray(inputs["bk"], np.float32),
        np.asarray(inputs["Wv"], np.float32), np.asarray(inputs["bv"], np.float32),
        np.asarray(inputs["Wo"], np.float32), np.asarray(inputs["bo"], np.float32),
    )
    nc = _get_nc()
    res = run_bass_kernel_spmd(nc, in_maps, list(range(8)), trace=trace, **spmd_kwargs)
    out = np.stack([res.results[i]["out"] for i in range(8)]).astype(np.float32)
    return out, res


def kernel(**inputs):
    out, _ = run(inputs, trace=False)
    return out

